# revision 57
# baseline (speedup 1.0000x reference)
"""Bilateral slice apply (HDRNet) Trainium2 Bass kernel — bf16 pair-packed.

Problem shapes (hardcoded):
  grid:  [4, 12, 8, 16, 16] f32   (B, (NIN+1)*NOUT, GD, GH, GW)
  guide: [4, 1, 1024, 1024] f32   in [0, 1)
  image: [4, 3, 1024, 1024] f32
  out:   [4, 3, 1024, 1024] f32

Sharding: 8 cores = (batch b = k//2, y-half h = k%2).  Each core computes
out[b, :, 512h:512h+512, :] from its guide/image shard and batch-b grid.

Algorithm (per core, per 128-row band):
  - y-interp on PE: gy[128, 12*8*64] = Ay_band.T @ grid_r (bf16 in, f32
    psum, bf16 out).  grid_r columns are (c, d, s', t): for 32-px segment
    s' in [0,32), t=0 holds the left and t=1 the right clamped x-corner
    grid value, so each pixel's (L, R) pair is stride-1 adjacent.
  - tent weights: guide stays f32 (z = 8*guide amplifies rounding);
    per depth d: Act Abs -> Act Relu written pair-duplicated -> DVE
    multiply by interleaved (wx0, wx1) constant, giving
    wzp_d[128, 2048] bf16 = (wz_d*wx0, wz_d*wx1) pairs.
  - products: per channel c, per d: one DVE/GpSimd bf16 tensor_mul of
    wzp_d against a (L, R) stride-1 pair view of gy.  All operand APs
    end in a stride-1 length-2 dim, so DVE runs in 2x_1p mode (2 elem/
    lane/cycle).
  - accumulate: PE identity matmuls sum the 16 products (8 d x 2 parity)
    per channel into f32 PSUM (2 x 512-col banks).
  - apply: C psum -> bf16 (Act), T_oj = C_oj * img_j (DVE, packed),
    PE-accumulates T slabs + bias into out psum, Act copies to f32,
    DMA out.  Output stays f32 end to end.

Scheduling: band b+1's Act-side weight build is issued before band b's
compute; the 8 DVE premults for band b+1 are interleaved into band b's
product stream (GpSimd-consumed depths first) so no engine stalls at
band boundaries.
"""

import os
import sys
import numpy as np

for _p in ("/opt/trn_rl_repo", "/root/.axon_site/_ro/trn_rl_repo"):
    if _p not in sys.path and os.path.isdir(_p):
        sys.path.insert(0, _p)

from contextlib import ExitStack  # noqa: E402

import ml_dtypes  # noqa: E402

import concourse.bass as bass  # noqa: E402
import concourse.tile as tile  # noqa: E402
from concourse import bacc, mybir  # noqa: E402
from concourse.bass_utils import run_bass_kernel_spmd  # noqa: E402

F32 = mybir.dt.float32
BF16 = mybir.dt.bfloat16
AF = mybir.ActivationFunctionType
ALU = mybir.AluOpType
BFNP = ml_dtypes.bfloat16

B, NIN, NOUT = 4, 3, 3
C = (NIN + 1) * NOUT  # 12
GD, GH, GW = 8, 16, 16
H, W = 1024, 1024
HS = H // 2          # rows per core (y-half)
NBAND = HS // 128    # 4 bands of 128 rows
XT = 1024            # slot = x, no padding
NS32 = 32            # 32-px segments
SEGW = 2 * NS32      # 64 table cols per (c,d): (L,R) interleaved
NCOLG = C * GD * SEGW  # 6144 gy columns
PAIRW = 2 * XT       # 2048

_cached = {}


def _host_consts():
    gyc_ = (np.arange(H) + 0.5) * (GH / H) - 0.5
    gyc = np.clip(gyc_, 0.0, GH - 1)
    idx = np.arange(GH)
    ay = np.maximum(1.0 - np.abs(gyc[:, None] - idx[None, :]), 0.0)
    ay_t0 = np.ascontiguousarray(ay[:HS].T).astype(BFNP)
    ay_t1 = np.ascontiguousarray(ay[HS:].T).astype(BFNP)

    # wx01: interleaved (wx0, wx1) per pixel x.
    gx = (np.arange(W) + 0.5) * (GW / W) - 0.5
    frac = (gx - np.floor(gx)).astype(np.float32)
    wx01 = np.empty((PAIRW,), np.float32)
    wx01[0::2] = 1.0 - frac
    wx01[1::2] = frac
    wx01c = np.broadcast_to(wx01, (128, PAIRW)).astype(BFNP).copy()

    dvals = np.concatenate([-np.arange(GD, dtype=np.float32),
                            np.array([-0.5, float(GD - 1)], np.float32)])
    dneg = np.broadcast_to(dvals, (128, GD + 2)).copy()
    eye = np.eye(128, dtype=np.float32).astype(BFNP)
    return ay_t0, ay_t1, wx01c, dneg, eye


def _relayout_grid(grid_b):
    """grid_b [12, 8, 16, 16] f32 -> [16(gh), 6144] bf16.

    col((c,d,s',t)) = (c*8+d)*64 + 2*s' + t.  For 32-px segment s'
    (pixels [32s', 32s'+32)), the original 64-px cell is
    s = (s'+1)//2, left corner G[clip(s-1)], right corner G[clip(s)].
    """
    s = (np.arange(NS32) + 1) // 2
    li = np.clip(s - 1, 0, GW - 1)
    ri = np.clip(s, 0, GW - 1)
    cols = np.empty((SEGW,), np.int64)
    cols[0::2] = li
    cols[1::2] = ri
    gp = grid_b[:, :, :, cols]                       # [12, 8, 16, 64]
    gr = gp.transpose(2, 0, 1, 3).reshape(GH, NCOLG)
    return np.ascontiguousarray(gr).astype(BFNP)


# products on GpSimd: d=7 always, d=6 for the first GP6 of the 12 channels
GP6 = 11
# weight-build order: GpSimd-consumed depths first
D_ORDER = (6, 7, 0, 1, 2, 3, 4, 5)


def _build_nc():
    nc = bacc.Bacc("TRN2", target_bir_lowering=False, debug=False,
                   num_devices=8)

    grid_r = nc.dram_tensor("grid_r", [GH, NCOLG], BF16, kind="ExternalInput").ap()
    guide_d = nc.dram_tensor("guide", [HS, W], F32, kind="ExternalInput").ap()
    img_d = nc.dram_tensor("img", [NIN * HS, W], BF16, kind="ExternalInput").ap()
    ay_d = nc.dram_tensor("ay_t", [GH, HS], BF16, kind="ExternalInput").ap()
    wx01_d = nc.dram_tensor("wx01", [128, PAIRW], BF16, kind="ExternalInput").ap()
    dneg_d = nc.dram_tensor("dneg", [128, GD + 2], F32, kind="ExternalInput").ap()
    eye_d = nc.dram_tensor("eye", [128, 128], BF16, kind="ExternalInput").ap()
    out_d = nc.dram_tensor("out", [NOUT * HS, W], F32, kind="ExternalOutput").ap()

    with tile.TileContext(nc) as tc, ExitStack() as ctx:
        cpool = ctx.enter_context(tc.tile_pool(name="consts", bufs=1))
        gy_pool = ctx.enter_context(tc.tile_pool(name="gy", bufs=2))
        ps_pool = ctx.enter_context(tc.tile_pool(name="ps", bufs=2, space="PSUM"))
        io_pool = ctx.enter_context(tc.tile_pool(name="io", bufs=2))
        wz_pool = ctx.enter_context(tc.tile_pool(name="wz", bufs=2))
        acc_pool = ctx.enter_context(tc.tile_pool(name="acc", bufs=2))

        ay_sb = cpool.tile([GH, HS], BF16, name="ay_sb")
        nc.sync.dma_start(ay_sb[:], ay_d[:, :])
        grid_sb = cpool.tile([GH, NCOLG], BF16, name="grid_sb")
        nc.sync.dma_start(grid_sb[:], grid_r[:, :])
        wx01_sb = cpool.tile([128, PAIRW], BF16, name="wx01_sb")
        nc.sync.dma_start(wx01_sb[:], wx01_d[:, :])
        dneg_sb = cpool.tile([128, GD + 2], F32, name="dneg_sb")
        nc.sync.dma_start(dneg_sb[:], dneg_d[:, :])
        eye_sb = cpool.tile([128, 128], BF16, name="eye_sb")
        nc.sync.dma_start(eye_sb[:], eye_d[:, :])

        def tent(state, d):
            gzc, wzdups = state[4], state[1]
            ad = wz_pool.tile([128, XT], F32, name=f"ad{d}", tag="ad")
            nc.scalar.activation(ad[:], gzc[:], AF.Abs,
                                 bias=dneg_sb[:, d:d + 1], scale=1.0)
            wzdup = wz_pool.tile([128, PAIRW], BF16, name=f"wzdup{d}",
                                 tag=f"wzdup{d}", bufs=1)
            in_ap = bass.AP(ad.tensor, ad.offset,
                            [[XT, 128], [1, XT], [0, 2]])
            out_ap = bass.AP(wzdup.tensor, wzdup.offset,
                             [[PAIRW, 128], [2, XT], [1, 2]])
            nc.scalar.activation(out_ap, in_ap, AF.Relu,
                                 bias=1.0, scale=-1.0)
            wzdups[d] = wzdup

        def gy_chunk(state, band, i):
            y0 = band * 128
            gy = state[0]
            off = i * 512
            w = min(512, NCOLG - off)
            ps = ps_pool.tile([128, 512], F32, name="gyps", tag="gyps",
                              bufs=2)
            nc.tensor.matmul(ps[:, :w], ay_sb[:, y0:y0 + 128],
                             grid_sb[:, off:off + w],
                             start=True, stop=True)
            nc.scalar.copy(gy[:, off:off + w], ps[:, :w])

        def build_weights(band):
            y0 = band * 128
            # ---- guide (f32) -> clamped z coordinate ----
            guide_t = io_pool.tile([128, XT], F32, name="guide_t", tag="guide")
            nc.sync.dma_start(guide_t[:], guide_d[y0:y0 + 128, :])
            gzf = wz_pool.tile([128, XT], F32, name="gzf", tag="gzf", bufs=1)
            nc.scalar.activation(gzf[:], guide_t[:], AF.Relu,
                                 bias=dneg_sb[:, GD:GD + 1], scale=float(GD))
            gzv = wz_pool.tile([128, XT], F32, name="gzv", tag="gzv", bufs=1)
            nc.scalar.activation(gzv[:], gzf[:], AF.Relu,
                                 bias=dneg_sb[:, GD + 1:GD + 2], scale=-1.0)
            gzc = wz_pool.tile([128, XT], F32, name="gzc", tag="gzc", bufs=1)
            nc.scalar.activation(gzc[:], gzv[:], AF.Copy,
                                 bias=float(GD - 1), scale=-1.0)

            gy = gy_pool.tile([128, NCOLG], BF16, name="gy")

            # ---- image tiles (bf16) ----
            imgt = []
            for j in range(NIN):
                it = io_pool.tile([128, XT], BF16, name=f"img{j}", tag=f"img{j}")
                nc.sync.dma_start(it[:],
                                  img_d[j * HS + y0:j * HS + y0 + 128, :])
                imgt.append(it)
            return [gy, [None] * GD, imgt, [None] * GD, gzc]

        def finish_weights(state, band):
            # startup path: GpSimd-critical pieces first
            for d in D_ORDER[:2]:
                tent(state, d)
                premult(state, d)
            gy_chunk(state, band, 0)
            for d in D_ORDER[2:4]:
                tent(state, d)
                premult(state, d)
            for i in range(1, (NCOLG + 511) // 512):
                gy_chunk(state, band, i)
            for d in D_ORDER[4:]:
                tent(state, d)
                premult(state, d)

        def premult(state, d):
            wzdups = state[1]
            wp = wz_pool.tile([128, PAIRW], BF16, name=f"wzp{d}",
                              tag=f"wzp{d}", bufs=2 if d in (6, 7) else 1)
            nc.vector.tensor_mul(wp[:], wzdups[d][:], wx01_sb[:])
            state[3][d] = wp

        def compute_band(band, state, nxt):
            gy, _, imgt, wzp = state[0], state[1], state[2], state[3]
            y0 = band * 128
            slices = [(0, 512), (512, 512)]

            for o in range(NOUT):
                tslabs = [None] * (NIN + 1)
                for j in range(NIN + 1):
                    c = o * 4 + j
                    oj = c
                    if nxt is not None:
                        gy_chunk(nxt, band + 1, oj)
                        if oj < GD:
                            tent(nxt, D_ORDER[oj])
                            if oj < 2:
                                premult(nxt, D_ORDER[oj])

                    def pv(t):
                        return bass.AP(t.tensor, t.offset,
                                       [[PAIRW, 128], [SEGW, NS32],
                                        [2, NS32], [1, 2]])

                    def gv(cc, d):
                        base = (cc * GD + d) * SEGW
                        return bass.AP(gy.tensor, gy.offset + base,
                                       [[NCOLG, 128], [2, NS32],
                                        [0, NS32], [1, 2]])

                    gp_ds = (6, 7) if oj < GP6 else (7,)
                    prods = []
                    for d in range(GD):
                        if d in gp_ds:
                            t = acc_pool.tile([128, PAIRW], BF16, name="tG",
                                              tag="tG", bufs=5)
                            nc.gpsimd.tensor_mul(pv(t), pv(wzp[d]), gv(c, d))
                        else:
                            t = acc_pool.tile([128, PAIRW], BF16, name="tV",
                                              tag="tV", bufs=4)
                            nc.vector.tensor_mul(pv(t), pv(wzp[d]), gv(c, d))
                        prods.append(t)

                    cacc = ps_pool.tile([128, 1024], F32, name="cacc",
                                        tag="cacc", bufs=2)
                    n = len(prods)
                    for i, t in enumerate(prods):
                        for par in range(2):
                            for xoff, tw in slices:
                                bv = bass.AP(t.tensor,
                                             t.offset + 2 * xoff + par,
                                             [[PAIRW, 128], [2, tw]])
                                nc.tensor.matmul(
                                    cacc[:, xoff:xoff + tw],
                                    eye_sb[:], bv,
                                    start=(i == 0 and par == 0),
                                    stop=(i == n - 1 and par == 1),
                                )
                    cbf = acc_pool.tile([128, XT], BF16, name="cbf",
                                        tag="cbf", bufs=2)
                    nc.scalar.copy(cbf[:], cacc[:])
                    if j < NIN:
                        tt = acc_pool.tile([128, XT], BF16, name="tt",
                                           tag="tt", bufs=3)
                        nc.vector.tensor_mul(tt[:], cbf[:], imgt[j][:])
                        tslabs[j] = tt
                    else:
                        tslabs[j] = cbf

                obf = io_pool.tile([128, XT], F32, name=f"obf{o}",
                                   tag="obf", bufs=2)
                for xoff, tw in slices:
                    ops = ps_pool.tile([128, 512], F32, name="ops",
                                       tag="gyps", bufs=2)
                    for k, ts in enumerate(tslabs):
                        nc.tensor.matmul(ops[:, :tw], eye_sb[:],
                                         ts[:, xoff:xoff + tw],
                                         start=(k == 0), stop=(k == 3))
                    nc.scalar.copy(obf[:, xoff:xoff + tw], ops[:, :tw])
                nc.sync.dma_start(out_d[o * HS + y0:o * HS + y0 + 128, :],
                                  obf[:])
            if nxt is not None:
                for d in D_ORDER[2:]:
                    premult(nxt, d)

        prev = None
        for band in range(NBAND):
            cur = build_weights(band)
            if prev is None:
                finish_weights(cur, band)
            if prev is not None:
                compute_band(band - 1, prev, cur)
            prev = cur
        compute_band(NBAND - 1, prev, None)

    nc.compile()
    return nc


def _get_nc():
    if "nc" not in _cached:
        _cached["nc"] = _build_nc()
    return _cached["nc"]


def kernel(grid, guide, image):
    grid = np.asarray(grid, dtype=np.float32)
    guide = np.asarray(guide, dtype=np.float32)
    image = np.asarray(image, dtype=np.float32)

    nc = _get_nc()
    ay_t0, ay_t1, wx01c, dneg, eye = _host_consts()
    ay_halves = (ay_t0, ay_t1)

    grid_rp = [_relayout_grid(grid[b]) for b in range(B)]
    image_bf = image.astype(BFNP)

    in_maps = []
    for k in range(8):
        b, h = k // 2, k % 2
        in_maps.append({
            "grid_r": grid_rp[b],
            "guide": np.ascontiguousarray(guide[b, 0, h * HS:(h + 1) * HS, :]),
            "img": np.ascontiguousarray(
                image_bf[b, :, h * HS:(h + 1) * HS, :]).reshape(NIN * HS, W),
            "ay_t": ay_halves[h],
            "wx01": wx01c,
            "dneg": dneg,
            "eye": eye,
        })

    res = run_bass_kernel_spmd(nc, in_maps, core_ids=list(range(8)))

    out = np.empty((B, NOUT, H, W), np.float32)
    for k in range(8):
        b, h = k // 2, k % 2
        out[b, :, h * HS:(h + 1) * HS, :] = \
            res.results[k]["out"].reshape(NOUT, HS, W).astype(np.float32)
    return out


# revision 62
# speedup vs baseline: 1.0116x; 1.0116x over previous
"""Bilateral slice apply (HDRNet) Trainium2 Bass kernel — bf16 pair-packed.

Problem shapes (hardcoded):
  grid:  [4, 12, 8, 16, 16] f32   (B, (NIN+1)*NOUT, GD, GH, GW)
  guide: [4, 1, 1024, 1024] f32   in [0, 1)
  image: [4, 3, 1024, 1024] f32
  out:   [4, 3, 1024, 1024] f32

Sharding: 8 cores = (batch b = k//2, y-half h = k%2).  Each core computes
out[b, :, 512h:512h+512, :] from its guide/image shard and batch-b grid.

Algorithm (per core, per 128-row band):
  - y-interp on PE: gy[128, 12*8*64] = Ay_band.T @ grid_r (bf16 in, f32
    psum, bf16 out).  grid_r columns are (c, d, s', t): for 32-px segment
    s' in [0,32), t=0 holds the left and t=1 the right clamped x-corner
    grid value, so each pixel's (L, R) pair is stride-1 adjacent.
  - tent weights: guide stays f32 (z = 8*guide amplifies rounding);
    per depth d: Act Abs -> Act Relu written pair-duplicated -> DVE
    multiply by interleaved (wx0, wx1) constant, giving
    wzp_d[128, 2048] bf16 = (wz_d*wx0, wz_d*wx1) pairs.
  - products: per channel c, per d: one DVE/GpSimd bf16 tensor_mul of
    wzp_d against a (L, R) stride-1 pair view of gy.  All operand APs
    end in a stride-1 length-2 dim, so DVE runs in 2x_1p mode (2 elem/
    lane/cycle).
  - accumulate: PE identity matmuls sum the 16 products (8 d x 2 parity)
    per channel into f32 PSUM (2 x 512-col banks).
  - apply: C psum -> bf16 (Act), T_oj = C_oj * img_j (DVE, packed),
    PE-accumulates T slabs + bias into out psum, Act copies to f32,
    DMA out.  Output stays f32 end to end.

Scheduling: band b+1's Act-side weight build is issued before band b's
compute; the 8 DVE premults for band b+1 are interleaved into band b's
product stream (GpSimd-consumed depths first) so no engine stalls at
band boundaries.
"""

import os
import sys
import numpy as np

for _p in ("/opt/trn_rl_repo", "/root/.axon_site/_ro/trn_rl_repo"):
    if _p not in sys.path and os.path.isdir(_p):
        sys.path.insert(0, _p)

from contextlib import ExitStack  # noqa: E402

import ml_dtypes  # noqa: E402

import concourse.bass as bass  # noqa: E402
import concourse.tile as tile  # noqa: E402
from concourse import bacc, mybir  # noqa: E402
from concourse.bass_utils import run_bass_kernel_spmd  # noqa: E402

F32 = mybir.dt.float32
BF16 = mybir.dt.bfloat16
AF = mybir.ActivationFunctionType
ALU = mybir.AluOpType
BFNP = ml_dtypes.bfloat16

B, NIN, NOUT = 4, 3, 3
C = (NIN + 1) * NOUT  # 12
GD, GH, GW = 8, 16, 16
H, W = 1024, 1024
HS = H // 2          # rows per core (y-half)
NBAND = HS // 128    # 4 bands of 128 rows
XT = 1024            # slot = x, no padding
NS32 = 32            # 32-px segments
SEGW = 2 * NS32      # 64 table cols per (c,d): (L,R) interleaved
NCOLG = C * GD * SEGW  # 6144 gy columns
PAIRW = 2 * XT       # 2048

_cached = {}


def _host_consts():
    gyc_ = (np.arange(H) + 0.5) * (GH / H) - 0.5
    gyc = np.clip(gyc_, 0.0, GH - 1)
    idx = np.arange(GH)
    ay = np.maximum(1.0 - np.abs(gyc[:, None] - idx[None, :]), 0.0)
    ay_t0 = np.ascontiguousarray(ay[:HS].T).astype(BFNP)
    ay_t1 = np.ascontiguousarray(ay[HS:].T).astype(BFNP)

    # wx01: interleaved (wx0, wx1) per pixel x.
    gx = (np.arange(W) + 0.5) * (GW / W) - 0.5
    frac = (gx - np.floor(gx)).astype(np.float32)
    wx01 = np.empty((PAIRW,), np.float32)
    wx01[0::2] = 1.0 - frac
    wx01[1::2] = frac
    wx01c = np.broadcast_to(wx01, (128, PAIRW)).astype(BFNP).copy()

    dvals = np.concatenate([-np.arange(GD, dtype=np.float32),
                            np.array([-0.5, float(GD - 1)], np.float32)])
    dneg = np.broadcast_to(dvals, (128, GD + 2)).copy()
    eye = np.eye(128, dtype=np.float32).astype(BFNP)
    return ay_t0, ay_t1, wx01c, dneg, eye


def _relayout_grid(grid_b):
    """grid_b [12, 8, 16, 16] f32 -> [16(gh), 6144] bf16.

    col((c,d,s',t)) = (c*8+d)*64 + 2*s' + t.  For 32-px segment s'
    (pixels [32s', 32s'+32)), the original 64-px cell is
    s = (s'+1)//2, left corner G[clip(s-1)], right corner G[clip(s)].
    """
    s = (np.arange(NS32) + 1) // 2
    li = np.clip(s - 1, 0, GW - 1)
    ri = np.clip(s, 0, GW - 1)
    cols = np.empty((SEGW,), np.int64)
    cols[0::2] = li
    cols[1::2] = ri
    gp = grid_b[:, :, :, cols]                       # [12, 8, 16, 64]
    gr = gp.transpose(2, 0, 1, 3).reshape(GH, NCOLG)
    return np.ascontiguousarray(gr).astype(BFNP)


# products on GpSimd: d=7 always, d=6 for the first GP6 of the 12 channels
GP6 = 11
# weight-build order: GpSimd-consumed depths first
D_ORDER = (6, 7, 0, 1, 2, 3, 4, 5)


def _build_nc():
    nc = bacc.Bacc("TRN2", target_bir_lowering=False, debug=False,
                   num_devices=8)

    grid_r = nc.dram_tensor("grid_r", [GH, NCOLG], BF16, kind="ExternalInput").ap()
    guide_d = nc.dram_tensor("guide", [HS, W], F32, kind="ExternalInput").ap()
    img_d = nc.dram_tensor("img", [NIN * HS, W], BF16, kind="ExternalInput").ap()
    ay_d = nc.dram_tensor("ay_t", [GH, HS], BF16, kind="ExternalInput").ap()
    wx01_d = nc.dram_tensor("wx01", [128, PAIRW], BF16, kind="ExternalInput").ap()
    dneg_d = nc.dram_tensor("dneg", [128, GD + 2], F32, kind="ExternalInput").ap()
    eye_d = nc.dram_tensor("eye", [128, 128], BF16, kind="ExternalInput").ap()
    out_d = nc.dram_tensor("out", [NOUT * HS, W], F32, kind="ExternalOutput").ap()

    with tile.TileContext(nc) as tc, ExitStack() as ctx:
        cpool = ctx.enter_context(tc.tile_pool(name="consts", bufs=1))
        gy_pool = ctx.enter_context(tc.tile_pool(name="gy", bufs=2))
        ps_pool = ctx.enter_context(tc.tile_pool(name="ps", bufs=2, space="PSUM"))
        io_pool = ctx.enter_context(tc.tile_pool(name="io", bufs=2))
        wz_pool = ctx.enter_context(tc.tile_pool(name="wz", bufs=2))
        acc_pool = ctx.enter_context(tc.tile_pool(name="acc", bufs=2))

        ay_sb = cpool.tile([GH, HS], BF16, name="ay_sb")
        nc.sync.dma_start(ay_sb[:], ay_d[:, :])
        grid_sb = cpool.tile([GH, NCOLG], BF16, name="grid_sb")
        nc.sync.dma_start(grid_sb[:], grid_r[:, :])
        wx01_sb = cpool.tile([128, PAIRW], BF16, name="wx01_sb")
        nc.sync.dma_start(wx01_sb[:], wx01_d[:, :])
        dneg_sb = cpool.tile([128, GD + 2], F32, name="dneg_sb")
        nc.sync.dma_start(dneg_sb[:], dneg_d[:, :])
        eye_sb = cpool.tile([128, 128], BF16, name="eye_sb")
        nc.sync.dma_start(eye_sb[:], eye_d[:, :])

        def tent(state, d):
            gzc, wzdups = state[4], state[1]
            ad = wz_pool.tile([128, XT], F32, name=f"ad{d}", tag="ad")
            # clamp(z,0,7) only matters for the edge tents: |clamp(z)-0| =
            # relu(z) and |clamp(z)-7| = relu(7-z); interior d use |z-d|.
            if d == 0:
                nc.scalar.activation(ad[:], gzc[:], AF.Relu,
                                     bias=0.0, scale=1.0)
            elif d == GD - 1:
                nc.scalar.activation(ad[:], gzc[:], AF.Relu,
                                     bias=dneg_sb[:, GD + 1:GD + 2],
                                     scale=-1.0)
            else:
                nc.scalar.activation(ad[:], gzc[:], AF.Abs,
                                     bias=dneg_sb[:, d:d + 1], scale=1.0)
            wzdup = wz_pool.tile([128, PAIRW], BF16, name=f"wzdup{d}",
                                 tag=f"wzdup{d}", bufs=1)
            in_ap = bass.AP(ad.tensor, ad.offset,
                            [[XT, 128], [1, XT], [0, 2]])
            out_ap = bass.AP(wzdup.tensor, wzdup.offset,
                             [[PAIRW, 128], [2, XT], [1, 2]])
            nc.scalar.activation(out_ap, in_ap, AF.Relu,
                                 bias=1.0, scale=-1.0)
            wzdups[d] = wzdup

        def gy_chunk(state, band, i):
            y0 = band * 128
            gy = state[0]
            off = i * 512
            w = min(512, NCOLG - off)
            ps = ps_pool.tile([128, 512], F32, name="gyps", tag="gyps",
                              bufs=2)
            nc.tensor.matmul(ps[:, :w], ay_sb[:, y0:y0 + 128],
                             grid_sb[:, off:off + w],
                             start=True, stop=True)
            nc.scalar.copy(gy[:, off:off + w], ps[:, :w])

        def build_weights(band):
            y0 = band * 128
            # ---- guide (f32) -> clamped z coordinate ----
            guide_t = io_pool.tile([128, XT], F32, name="guide_t", tag="guide")
            nc.sync.dma_start(guide_t[:], guide_d[y0:y0 + 128, :])
            gzc = wz_pool.tile([128, XT], F32, name="gzc", tag="gzc", bufs=1)
            nc.scalar.activation(gzc[:], guide_t[:], AF.Copy,
                                 bias=-0.5, scale=float(GD))

            gy = gy_pool.tile([128, NCOLG], BF16, name="gy")

            # ---- image tiles (bf16) ----
            imgt = []
            for j in range(NIN):
                it = io_pool.tile([128, XT], BF16, name=f"img{j}", tag=f"img{j}")
                nc.sync.dma_start(it[:],
                                  img_d[j * HS + y0:j * HS + y0 + 128, :])
                imgt.append(it)
            return [gy, [None] * GD, imgt, [None] * GD, gzc]

        def finish_weights(state, band):
            # startup path: GpSimd-critical pieces first
            for d in D_ORDER[:2]:
                tent(state, d)
                premult(state, d)
            gy_chunk(state, band, 0)
            for d in D_ORDER[2:4]:
                tent(state, d)
                premult(state, d)
            for i in range(1, (NCOLG + 511) // 512):
                gy_chunk(state, band, i)
            for d in D_ORDER[4:]:
                tent(state, d)
                premult(state, d)

        def premult(state, d):
            wzdups = state[1]
            wp = wz_pool.tile([128, PAIRW], BF16, name=f"wzp{d}",
                              tag=f"wzp{d}", bufs=2 if d in (6, 7) else 1)
            nc.vector.tensor_mul(wp[:], wzdups[d][:], wx01_sb[:])
            state[3][d] = wp

        def compute_band(band, state, nxt):
            gy, _, imgt, wzp = state[0], state[1], state[2], state[3]
            y0 = band * 128
            slices = [(0, 512), (512, 512)]

            for o in range(NOUT):
                tslabs = [None] * (NIN + 1)
                for j in range(NIN + 1):
                    c = o * 4 + j
                    oj = c
                    if nxt is not None:
                        gy_chunk(nxt, band + 1, oj)
                        if oj < GD:
                            tent(nxt, D_ORDER[oj])
                            if oj < 2:
                                premult(nxt, D_ORDER[oj])

                    def pv(t):
                        return bass.AP(t.tensor, t.offset,
                                       [[PAIRW, 128], [SEGW, NS32],
                                        [2, NS32], [1, 2]])

                    def gv(cc, d):
                        base = (cc * GD + d) * SEGW
                        return bass.AP(gy.tensor, gy.offset + base,
                                       [[NCOLG, 128], [2, NS32],
                                        [0, NS32], [1, 2]])

                    gp_ds = (6, 7) if oj < GP6 else (7,)
                    prods = []
                    for d in range(GD):
                        if d in gp_ds:
                            t = acc_pool.tile([128, PAIRW], BF16, name="tG",
                                              tag="tG", bufs=5)
                            nc.gpsimd.tensor_mul(pv(t), pv(wzp[d]), gv(c, d))
                        else:
                            t = acc_pool.tile([128, PAIRW], BF16, name="tV",
                                              tag="tV", bufs=6)
                            nc.vector.tensor_mul(pv(t), pv(wzp[d]), gv(c, d))
                        prods.append(t)

                    cacc = ps_pool.tile([128, 1024], F32, name="cacc",
                                        tag="cacc", bufs=2)
                    n = len(prods)
                    for i, t in enumerate(prods):
                        for par in range(2):
                            for xoff, tw in slices:
                                bv = bass.AP(t.tensor,
                                             t.offset + 2 * xoff + par,
                                             [[PAIRW, 128], [2, tw]])
                                nc.tensor.matmul(
                                    cacc[:, xoff:xoff + tw],
                                    eye_sb[:], bv,
                                    start=(i == 0 and par == 0),
                                    stop=(i == n - 1 and par == 1),
                                )
                    cbf = acc_pool.tile([128, XT], BF16, name="cbf",
                                        tag="cbf", bufs=2)
                    nc.scalar.copy(cbf[:], cacc[:])
                    if j < NIN:
                        tt = acc_pool.tile([128, XT], BF16, name="tt",
                                           tag="tt", bufs=3)
                        nc.vector.tensor_mul(tt[:], cbf[:], imgt[j][:])
                        tslabs[j] = tt
                    else:
                        tslabs[j] = cbf

                obf = io_pool.tile([128, XT], F32, name=f"obf{o}",
                                   tag="obf", bufs=2)
                for xoff, tw in slices:
                    ops = ps_pool.tile([128, 512], F32, name="ops",
                                       tag="gyps", bufs=2)
                    for k, ts in enumerate(tslabs):
                        nc.tensor.matmul(ops[:, :tw], eye_sb[:],
                                         ts[:, xoff:xoff + tw],
                                         start=(k == 0), stop=(k == 3))
                    nc.scalar.copy(obf[:, xoff:xoff + tw], ops[:, :tw])
                nc.sync.dma_start(out_d[o * HS + y0:o * HS + y0 + 128, :],
                                  obf[:])
            if nxt is not None:
                for d in D_ORDER[2:]:
                    premult(nxt, d)

        prev = None
        for band in range(NBAND):
            cur = build_weights(band)
            if prev is None:
                finish_weights(cur, band)
            if prev is not None:
                compute_band(band - 1, prev, cur)
            prev = cur
        compute_band(NBAND - 1, prev, None)

    nc.compile()
    return nc


def _get_nc():
    if "nc" not in _cached:
        _cached["nc"] = _build_nc()
    return _cached["nc"]


def kernel(grid, guide, image):
    grid = np.asarray(grid, dtype=np.float32)
    guide = np.asarray(guide, dtype=np.float32)
    image = np.asarray(image, dtype=np.float32)

    nc = _get_nc()
    ay_t0, ay_t1, wx01c, dneg, eye = _host_consts()
    ay_halves = (ay_t0, ay_t1)

    grid_rp = [_relayout_grid(grid[b]) for b in range(B)]
    image_bf = image.astype(BFNP)

    in_maps = []
    for k in range(8):
        b, h = k // 2, k % 2
        in_maps.append({
            "grid_r": grid_rp[b],
            "guide": np.ascontiguousarray(guide[b, 0, h * HS:(h + 1) * HS, :]),
            "img": np.ascontiguousarray(
                image_bf[b, :, h * HS:(h + 1) * HS, :]).reshape(NIN * HS, W),
            "ay_t": ay_halves[h],
            "wx01": wx01c,
            "dneg": dneg,
            "eye": eye,
        })

    res = run_bass_kernel_spmd(nc, in_maps, core_ids=list(range(8)))

    out = np.empty((B, NOUT, H, W), np.float32)
    for k in range(8):
        b, h = k // 2, k % 2
        out[b, :, h * HS:(h + 1) * HS, :] = \
            res.results[k]["out"].reshape(NOUT, HS, W).astype(np.float32)
    return out
